# revision 2
# baseline (speedup 1.0000x reference)
"""Causal attention (B=4, L=2048, d_model=1024, d_k=d_v=128) on 8 TRN2 NeuronCores.

Sharding (SPMD — one program, per-core data):
  core c -> batch b = c//2, parity par = c%2.
  Core handles q-blocks j = 2k+par for slot k in 0..7 (128 rows each).
  X^T's column blocks are split by parity: xq pieces (this core's
  query-parity blocks, which are also half the keys) and xo pieces (the
  other parity's blocks).  Slot k attends key-slots 0..k of EACH parity —
  a uniform instruction stream across cores.  The causal boundary is
  uniform too: the triangular mask always lands on q-parity key-slot
  m == k (zeroed post-exp with a gpsimd affine_select, same on every
  core), while other-parity key-slot m == k is fully masked (even cores)
  or fully valid (odd cores) — expressed as a per-partition exp bias
  vector fed as data (0 or -1e4).
  Every core projects K/V for all 2048 rows of its batch (KV compute
  duplicated within a pair; no collectives).

Perf notes vs the original baseline:
  - All DRAM inputs are host-relaid so every DMA is 128 descriptors of
    >=2KB contiguous rows (was 1024x512B), cutting HWDGE issue+SDMA time.
  - Input DMAs are split across BOTH HWDGE queues (sync + scalar) so
    issue overhead overlaps; output DMAs go to sync (idle at the tail).
  - 1/sqrt(d_k) is folded into W_Q on the host.
  - The causal mask add (vector) is gone: exp runs straight off PSUM,
    boundary handling as described above.  This shortens the
    matmul->exp PSUM bank lifetime, removing PE stalls.
  - Scores are computed TRANSPOSED: S^T[key, q] = K^T_blk.T @ Q^T, one
    N<=512 matmul per (parity, key-slot, slot-group).  exp() writes A^T
    straight to SBUF (bf16).
  - V is augmented with a ones column; Z_aug = A^T.T @ [V | 1] yields the
    softmax denominator in column 128 for free.  Softmax skips the
    row-max subtraction (scores bounded ~|12|; exp is safe in f32).
"""

import os
import sys

sys.path.insert(0, "/opt/trn_rl_repo")
sys.path.insert(0, "/opt/trn_rl_repo/concourse")

import ml_dtypes
import numpy as np

import concourse.bass as bass  # noqa: F401
import concourse.mybir as mybir
import concourse.tile as tile
from concourse import bacc
from concourse.bass_utils import run_bass_kernel_spmd
from concourse.masks import make_identity

B, L, DM, DK, DV = 4, 2048, 1024, 128, 128
NB = L // 128   # 16 key blocks per batch
SLOTS = 8       # q-blocks per core
NCH = DM // 128  # 8 d_model chunks
SCALE = float(DK) ** -0.5
BIASMASK = -1e4  # exp(-1e4 + s) == 0.0 exactly in f32

F32 = mybir.dt.float32
BF16 = mybir.dt.bfloat16
NPBF16 = ml_dtypes.bfloat16

# xq/xo column pieces (in 128-col q-slots): (name, lo_slot, n_slots)
XQ_PIECES = [("xqa", 0, 2), ("xqb", 2, 2), ("xqc", 4, 4)]
XO_PIECES = [("xoa", 0, 4), ("xob", 4, 4)]


def build_nc():
    nc = bacc.Bacc()

    # ---- DRAM params (all host-relaid to be row-contiguous) ----
    def xpiece_ext(name, ns):
        return nc.declare_dram_parameter(name, [128, NCH * ns * 128], BF16,
                                         isOutput=False)

    xq_ext = {name: xpiece_ext(name, ns) for name, _, ns in XQ_PIECES}
    xo_ext = {name: xpiece_ext(name, ns) for name, _, ns in XO_PIECES}
    # weights pre-arranged on host to the SBUF chunk layout
    # [p, c*128+d] = W[c*128+p, d] so the DMA is fully contiguous
    wq_ext = nc.declare_dram_parameter("wq", [128, DM], BF16, isOutput=False)
    wk_ext = nc.declare_dram_parameter("wk", [128, DM], BF16, isOutput=False)
    wv_ext = nc.declare_dram_parameter("wv", [128, DM], BF16, isOutput=False)
    # per-partition exp bias for the other-parity boundary block:
    # col 0 = 0.0 (odd cores, fully valid) or -1e4 (even cores, masked)
    bias_ext = nc.declare_dram_parameter("biasv", [128, 8], F32, isOutput=False)
    # out[p, k*128+d] = Z[q-slot k, row p, d]; host reassembles
    out_ext = nc.declare_dram_parameter("out", [128, SLOTS * DV], F32,
                                        isOutput=True)

    with tile.TileContext(nc) as tc:
        with (
            tc.tile_pool(name="persist", bufs=1) as persist,
            tc.tile_pool(name="mm_ps", bufs=6, space="PSUM") as mm_ps,
            tc.tile_pool(name="z_ps", bufs=2, space="PSUM") as z_ps,
            tc.tile_pool(name="work", bufs=6) as work,
        ):
            # ---- constants ----
            ident = persist.tile([128, 128], BF16, tag="ident")
            make_identity(nc, ident)

            # ---- input DMAs: sync queue = xq pieces, scalar = the rest ----
            xq_sb, xo_sb = {}, {}

            def load_piece(store, ext, name, ns, eng):
                t = persist.tile([128, NCH, ns * 128], BF16, tag=name, name=name)
                eng.dma_start(out=t[:], in_=ext[name].rearrange(
                    "p (c w) -> p c w", w=ns * 128))
                store[name] = t

            w_sb = {}

            def load_w(name, ext):
                t = persist.tile([128, NCH, 128], BF16, tag=name, name=name)
                nc.scalar.dma_start(
                    out=t[:], in_=ext.rearrange("p (c d) -> p c d", d=128))
                w_sb[name] = t

            load_w("wq", wq_ext)
            load_piece(xq_sb, xq_ext, "xqa", 2, nc.sync)
            load_w("wk", wk_ext)
            load_piece(xq_sb, xq_ext, "xqb", 2, nc.sync)
            load_w("wv", wv_ext)
            bias_sb = persist.tile([128, 8], F32, tag="biasv")
            nc.scalar.dma_start(out=bias_sb[:], in_=bias_ext[:])
            load_piece(xq_sb, xq_ext, "xqc", 4, nc.sync)
            load_piece(xo_sb, xo_ext, "xoa", 4, nc.scalar)
            load_piece(xo_sb, xo_ext, "xob", 4, nc.scalar)

            # ---- persistent SBUF tensors ----
            # qt/kt: [128, slot*128] transposed projections (Q^T, K^T)
            qt = persist.tile([128, SLOTS * 128], BF16, tag="qt", name="qt")
            kt = [persist.tile([128, SLOTS * 128], BF16, tag=f"kt{sp}",
                               name=f"kt{sp}") for sp in range(2)]
            # V^T per (parity, group-of-4-slots)
            vt = {(sp, g): persist.tile([128, 512], BF16, tag=f"vt{sp}{g}",
                                        name=f"vt{sp}{g}")
                  for sp in range(2) for g in range(2)}
            # V augmented with ones column, [key, d_v + 1] per (parity, slot)
            v_aug = {sp: persist.tile([128, SLOTS, DV + 1], BF16,
                                      tag=f"va{sp}", name=f"va{sp}")
                     for sp in range(2)}
            for sp in range(2):
                nc.vector.memset(v_aug[sp][:, :, DV:DV + 1], 1.0)
            # A^T tiles: [key 128, 512 q] per (parity, key-slot, q-group)
            at = {}
            for sp in range(2):
                for m in range(SLOTS):
                    for g in range(2):
                        if m <= 4 * g + 3:
                            at[(sp, m, g)] = persist.tile(
                                [128, 512], BF16, tag=f"at{sp}_{m}_{g}",
                                name=f"at{sp}_{m}_{g}")
            # final Z staging (f32) per slot
            # (separate work tiles per k; DMA'd out on sync queue)

            # ---- projection: weight-stationary over a 512/256-col piece ----
            def proj(wname, piece, dst, dst_off):
                w = w_sb[wname]
                wd = piece.shape[-1]
                ps = mm_ps.tile([128, wd], F32, tag="mm", name=f"p{wname}")
                for c in range(NCH):
                    nc.tensor.matmul(
                        ps[:], w[:, c, :], piece[:, c, :],
                        start=(c == 0), stop=(c == NCH - 1),
                    )
                dslice = dst[:, dst_off:dst_off + wd]
                if wname == "wv":
                    # keep V^T copies off the Scalar engine (it owns the exps)
                    nc.vector.tensor_copy(dslice, ps[:])
                else:
                    nc.scalar.copy(dslice, ps[:])

            def proj_piece(wname, store, pname, dst, dst_off=None):
                name_lo = {n: lo for n, lo, _ in XQ_PIECES + XO_PIECES}
                off = name_lo[pname] * 128 if dst_off is None else dst_off
                proj(wname, store[pname], dst, off)

            # ---- V^T -> V transposes (PE) + copy into v_aug ----
            def vt_blocks(sp, ms):
                for m in ms:
                    vps = mm_ps.tile([128, 128], BF16, tag="mm", name="vps")
                    nc.tensor.transpose(
                        vps[:],
                        vt[(sp, m // 4)][:, (m % 4) * 128:(m % 4 + 1) * 128],
                        ident[:],
                    )
                    nc.vector.tensor_copy(v_aug[sp][:, m, 0:DV], vps[:])

            # ---- scores + exp (mask folded into exp / affine_select) ----
            def scores(sp, ms):
                for m in ms:
                    for g in range(2):
                        lo = max(m, 4 * g)
                        if lo > 4 * g + 3:
                            continue
                        a = lo - 4 * g
                        has_diag = 4 * g <= m <= 4 * g + 3
                        st = mm_ps.tile([128, 512], F32, tag="mm", name="st")
                        nc.tensor.matmul(
                            st[:, a * 128:512],
                            kt[sp][:, m * 128:(m + 1) * 128],
                            qt[:, (4 * g + a) * 128:(4 * g + 4) * 128],
                            start=True, stop=True,
                            skip_group_check=True,
                        )
                        dst = at[(sp, m, g)]
                        if not has_diag:
                            nc.scalar.activation(
                                dst[:, a * 128:512], st[:, a * 128:512],
                                mybir.ActivationFunctionType.Exp)
                        elif sp == 0:
                            # own parity: triangle on the diag block; exp all,
                            # then zero the strict lower triangle (key > q)
                            nc.scalar.activation(
                                dst[:, a * 128:512], st[:, a * 128:512],
                                mybir.ActivationFunctionType.Exp)
                            blk = dst[:, a * 128:(a + 1) * 128]
                            nc.gpsimd.affine_select(
                                out=blk, in_=blk,
                                compare_op=mybir.AluOpType.is_ge,
                                fill=0.0, base=0,
                                # keep where q - key >= 0
                                pattern=[[1, 128]], channel_multiplier=-1,
                            )
                        else:
                            # other parity: all-or-nothing by core parity,
                            # via per-partition exp bias (0 or -1e4)
                            nc.scalar.activation(
                                dst[:, a * 128:(a + 1) * 128],
                                st[:, a * 128:(a + 1) * 128],
                                mybir.ActivationFunctionType.Exp,
                                bias=bias_sb[:, 0:1])
                            if a < 3:
                                nc.scalar.activation(
                                    dst[:, (a + 1) * 128:512],
                                    st[:, (a + 1) * 128:512],
                                    mybir.ActivationFunctionType.Exp)

            # ---- A^T.T @ [V|1], normalize, store ----
            def av(ks):
                for k in ks:
                    g, q = k // 4, (k % 4) * 128
                    zp = z_ps.tile([128, DV + 1], F32, tag="z")
                    for m in range(k + 1):
                        for sp in range(2):
                            nc.tensor.matmul(
                                zp[:],
                                at[(sp, m, g)][:, q:q + 128],
                                v_aug[sp][:, m, :],
                                start=(m == 0 and sp == 0),
                                stop=(m == k and sp == 1),
                            )
                    rcp = work.tile([128, 1], F32, tag="rcp")
                    nc.vector.reciprocal(rcp[:], zp[:, DV:DV + 1])
                    z_sb = work.tile([128, DV], F32, tag="zout")
                    nc.vector.tensor_scalar_mul(z_sb[:], zp[:, 0:DV], rcp[:])
                    nc.sync.dma_start(
                        out=out_ext[:, k * DV:(k + 1) * DV], in_=z_sb[:])

            # ---- emission in stream-arrival order ----
            proj_piece("wq", xq_sb, "xqa", qt)
            proj_piece("wq", xq_sb, "xqb", qt)
            proj_piece("wk", xq_sb, "xqa", kt[0])
            proj_piece("wk", xq_sb, "xqb", kt[0])
            proj_piece("wv", xq_sb, "xqa", vt[(0, 0)], 0)
            proj_piece("wv", xq_sb, "xqb", vt[(0, 0)], 256)
            vt_blocks(0, range(0, 4))
            proj_piece("wq", xq_sb, "xqc", qt)
            scores(0, range(0, 4))
            proj_piece("wk", xq_sb, "xqc", kt[0])
            proj_piece("wv", xq_sb, "xqc", vt[(0, 1)], 0)
            vt_blocks(0, range(4, 8))
            scores(0, range(4, 8))
            proj_piece("wk", xo_sb, "xoa", kt[1])
            proj_piece("wv", xo_sb, "xoa", vt[(1, 0)], 0)
            vt_blocks(1, range(0, 4))
            scores(1, range(0, 4))
            av(range(0, 4))
            proj_piece("wk", xo_sb, "xob", kt[1])
            proj_piece("wv", xo_sb, "xob", vt[(1, 1)], 0)
            vt_blocks(1, range(4, 8))
            scores(1, range(4, 8))
            av(range(4, 8))

    nc.finalize()
    return nc


_NC = None


def _get_nc():
    global _NC
    if _NC is None:
        _NC = build_nc()
    return _NC


def _pieces(xt_chunks, pieces):
    """xt_chunks: [NCH, 128, 1024] (dm-chunk, dm-in-chunk, l). Returns dict
    name -> [128, NCH*w] with row p = [c0 cols, c1 cols, ...]."""
    out = {}
    for name, lo, ns in pieces:
        w = ns * 128
        sl = xt_chunks[:, :, lo * 128:lo * 128 + w]          # [NCH,128,w]
        out[name] = np.ascontiguousarray(
            sl.transpose(1, 0, 2).reshape(128, NCH * w))
    return out


def kernel(X, W_Q, W_K, W_V):
    X = np.asarray(X, np.float32)
    W_Q = np.asarray(W_Q, np.float32) * SCALE
    W_K = np.asarray(W_K, np.float32)
    W_V = np.asarray(W_V, np.float32)

    nc = _get_nc()

    def warr(W):
        return np.ascontiguousarray(
            W.astype(NPBF16).reshape(NCH, 128, DK).transpose(1, 0, 2)
            .reshape(128, NCH * DK))

    wq, wk, wv = warr(W_Q), warr(W_K), warr(W_V)
    bias_even = np.zeros((128, 8), np.float32)
    bias_even[:, 0] = BIASMASK
    bias_odd = np.zeros((128, 8), np.float32)

    in_maps = []
    for c in range(8):
        b, par = c // 2, c % 2
        xt = np.ascontiguousarray(X[b].T).astype(NPBF16)     # [DM, L]
        qcols = np.concatenate(
            [np.arange((2 * k + par) * 128, (2 * k + par + 1) * 128)
             for k in range(SLOTS)])
        ocols = np.concatenate(
            [np.arange((2 * k + 1 - par) * 128, (2 * k + 2 - par) * 128)
             for k in range(SLOTS)])
        xq_chunks = xt[:, qcols].reshape(NCH, 128, SLOTS * 128)
        xo_chunks = xt[:, ocols].reshape(NCH, 128, SLOTS * 128)
        m = {"wq": wq, "wk": wk, "wv": wv,
             "biasv": bias_odd if par else bias_even}
        m.update(_pieces(xq_chunks, XQ_PIECES))
        m.update(_pieces(xo_chunks, XO_PIECES))
        in_maps.append(m)

    res = run_bass_kernel_spmd(nc, in_maps, list(range(8)))

    Z = np.zeros((B, L, DV), np.float32)
    for c in range(8):
        b, par = c // 2, c % 2
        o = res.results[c]["out"]                            # [128, 8*128]
        for k in range(SLOTS):
            j = 2 * k + par
            Z[b, j * 128:(j + 1) * 128, :] = o[:, k * DV:(k + 1) * DV]
    return Z
